# revision 1
# baseline (speedup 1.0000x reference)
"""Trainium2 Bass kernel for nn_Conv2d_91311004713559 (LUT-conv / gnn_message_passing).

Math: per table t (2,073,600 of them), the reference computes a 2-input LUT
    out[b,t] = sum_c basis[b,t,c] * w[t,c],  basis = prod_j (1 + combo[c,j]*xk)/2
which algebraically reduces (Lagrange basis, K=2) to
    out[b,t] = a_t + b_t*x0 + c_t*x1 + d_t*x0*x1
with (a,b,c,d) a fixed 4x4 linear transform of the truth-table weights.
Then tables reduce in groups of TPP=144 per output pixel.

Sharding: tables across the 8 NeuronCores by out-channel pair (expert-style per
the sharding hint); each core computes its own 2x900 output pixels end-to-end.

Split of work: this container's toolchain rejects every device-side gather
primitive (Q7 extended-ISA ops fail walrus codegen; IndirectCopy fails at the
runtime), so the batch-independent index gather is done host-side as input
marshaling, and the device kernel does all the streaming arithmetic: bf16
elementwise v = b*x0 + c*x1 + d*x0*x1 on DVE, fp32 segmented 144:1 reduction to
pixels, and the per-pixel bias (sum of a_t) add. Layout puts 15 pixels (2160
tables) on each partition so the pixel reduction never crosses partitions.
"""

import numpy as np
import ml_dtypes

# ---- static problem config (hardcoded per contract) ----
B = 16
IN_CH, OUT_CH = 16, 16
H, W = 32, 32
H_OUT = W_OUT = 30
POS = H_OUT * W_OUT            # 900
TPP = IN_CH * 3 * 3            # 144
T = OUT_CH * POS * TPP         # 2,073,600
N_CORES = 8
T_NC = T // N_CORES            # 259,200 tables / core (= 2 out-channels)
PIX_NC = 2 * POS               # 1800 pixels / core
PPP = 15                       # pixel slots per partition (128*15 = 1920 >= 1800)
PIX_PAD = 128 * PPP            # 1920
TAB_PP = PPP * TPP             # 2160 tables per partition
FREE = B * TAB_PP              # 34560 bf16 elems per partition per stream
BG = 4                         # batch group size for device tiling
GFREE = BG * TAB_PP            # 8640

_NC_CACHE = {}


def _patch_tile_drain_and_waits():
    """This env's walrus accepts at most one semaphore wait per instruction.
    Split Tile's end-of-kernel drain waits, and any other multi-wait
    instruction, onto single-wait InstNoOp's."""
    import concourse.mybir as mybir
    from concourse.tile import TileContext, ScopedClock

    if getattr(TileContext, "_ant_drain_patched", False):
        return

    def _drain_and_barrier(self, tick_clock, wait_clock):
        drain_inst = self.nc.sync.drain()
        wait_clock.add_sem_waits(
            drain_inst.ins, ScopedClock({None: tick_clock.global_clock})
        )
        si = drain_inst.ins.sync_info
        if si is not None and si.on_wait and len(si.on_wait) > 1:
            waits = list(si.on_wait)
            si.on_wait = waits[:1]
            for i in range(1, len(waits)):
                nop = self.nc.sync.nop(nofuse=True)
                nsi = nop.ins.sync_info
                if nsi is None:
                    nop.ins.sync_info = mybir.SyncInfo(
                        on_wait=waits[i : i + 1], on_update=[]
                    )
                else:
                    nsi.on_wait = waits[i : i + 1]
        self.nc.all_engine_barrier()
        popped = self.nc._tile_sem_poison_stack.pop()
        assert popped is self._sem_poison
        self.nc.clear_and_free_semaphores(list(self.sems.allocated().values()))
        self.nc.all_engine_barrier()

    TileContext._drain_and_barrier = _drain_and_barrier
    TileContext._ant_drain_patched = True


def _split_multi_waits(nc):
    import concourse.mybir as mybir

    for f in nc.m.functions:
        for blk in f.blocks:
            il = list(blk.instructions)
            out = []
            changed = False
            for ins in il:
                si = getattr(ins, "sync_info", None)
                if si is not None and si.on_wait and len(si.on_wait) > 1:
                    waits = list(si.on_wait)
                    for i in range(len(waits) - 1):
                        nop = mybir.InstNoOp(name=f"{ins.name}_ws{i}", ins=[], outs=[])
                        nop.engine = ins.engine
                        nop.sync_info = mybir.SyncInfo(
                            on_wait=waits[i : i + 1], on_update=[]
                        )
                        out.append(nop)
                    si.on_wait = waits[-1:]
                    changed = True
                out.append(ins)
            if changed:
                blk.instructions = out


def _build_device_kernel():
    """One SPMD NeuronCore program: streams x0/x1/coeff tiles, computes
    v = b*x0 + c*x1 + d*x0*x1 in-place on DVE (bf16), reduces 144:1 to fp32
    pixels, adds the per-pixel bias."""
    import concourse.bass as bass
    import concourse.mybir as mybir
    from concourse.tile import TileContext

    _patch_tile_drain_and_waits()

    F32 = mybir.dt.float32
    BF16 = mybir.dt.bfloat16
    nc = bass.Bass()

    x0_d = nc.dram_tensor("x0", [128, FREE], BF16, kind="ExternalInput")
    x1_d = nc.dram_tensor("x1", [128, FREE], BF16, kind="ExternalInput")
    cb_d = nc.dram_tensor("cb", [128, GFREE], BF16, kind="ExternalInput")
    cc_d = nc.dram_tensor("cc", [128, GFREE], BF16, kind="ExternalInput")
    cd_d = nc.dram_tensor("cd", [128, GFREE], BF16, kind="ExternalInput")
    bias_d = nc.dram_tensor("bias", [128, BG * PPP], F32, kind="ExternalInput")
    out_d = nc.dram_tensor("out", [128, B * PPP], F32, kind="ExternalOutput")

    add = mybir.AluOpType.add
    mult = mybir.AluOpType.mult

    with TileContext(nc) as tc:
        with (
            tc.tile_pool(name="coef", bufs=1) as cpool,
            tc.tile_pool(name="work", bufs=2) as wpool,
            tc.tile_pool(name="outp", bufs=2) as opool,
        ):
            cbt = cpool.tile([128, GFREE], BF16)
            nc.sync.dma_start(cbt[:], cb_d[:])
            cct = cpool.tile([128, GFREE], BF16)
            nc.sync.dma_start(cct[:], cc_d[:])
            cdt = cpool.tile([128, GFREE], BF16)
            nc.sync.dma_start(cdt[:], cd_d[:])
            biast = cpool.tile([128, BG * PPP], F32)
            nc.sync.dma_start(biast[:], bias_d[:])

            for g in range(B // BG):
                sl = slice(g * GFREE, (g + 1) * GFREE)
                x0t = wpool.tile([128, GFREE], BF16)
                nc.sync.dma_start(x0t[:], x0_d[:, sl])
                x1t = wpool.tile([128, GFREE], BF16)
                nc.sync.dma_start(x1t[:], x1_d[:, sl])
                m1 = wpool.tile([128, GFREE], BF16)
                # m1 = x0*x1
                nc.vector.tensor_tensor(m1[:], x0t[:], x1t[:], op=mult)
                # x1 <- c*x1 ; x0 <- b*x0 ; m1 <- d*m1   (in place)
                nc.vector.tensor_tensor(x1t[:], x1t[:], cct[:], op=mult)
                nc.vector.tensor_tensor(x0t[:], x0t[:], cbt[:], op=mult)
                nc.vector.tensor_tensor(m1[:], m1[:], cdt[:], op=mult)
                # v = m1 + x1 + x0   (into m1, then x0)
                nc.vector.tensor_tensor(m1[:], m1[:], x1t[:], op=add)
                nc.vector.tensor_tensor(m1[:], m1[:], x0t[:], op=add)
                # 144:1 segmented reduce to fp32 pixels
                red = opool.tile([128, BG * PPP], F32)
                v3 = m1[:].rearrange("p (k r) -> p k r", r=TPP)
                nc.vector.tensor_reduce(
                    red[:], v3, axis=mybir.AxisListType.X, op=add
                )
                outg = opool.tile([128, BG * PPP], F32)
                nc.vector.tensor_tensor(outg[:], red[:], biast[:], op=add)
                nc.sync.dma_start(
                    out_d[:, g * BG * PPP : (g + 1) * BG * PPP], outg[:]
                )

    _split_multi_waits(nc)
    return nc


def kernel(x, input_mask, weight):
    from concourse.bass_utils import run_bass_kernel_spmd

    x = np.asarray(x, dtype=np.float32)
    input_mask = np.asarray(input_mask)
    weight = np.asarray(weight, dtype=np.float32)

    # ---- host: batch-independent parameter preprocessing + marshaling ----
    lin = (
        input_mask[:, 0].astype(np.int64) * (H * W)
        + input_mask[:, 1].astype(np.int64) * W
        + input_mask[:, 2].astype(np.int64)
    )
    flat = x.reshape(B, IN_CH * H * W)
    gathered = flat[:, lin]                      # [B, 2T] host gather
    x0 = gathered[:, 0::2]                       # [B, T]
    x1 = gathered[:, 1::2]

    w0, w1, w2, w3 = weight[:, 0], weight[:, 1], weight[:, 2], weight[:, 3]
    ca = 0.25 * (w0 + w1 + w2 + w3)
    cb = 0.25 * (-w0 + w1 - w2 + w3)
    cc = 0.25 * (-w0 - w1 + w2 + w3)
    cd = 0.25 * (w0 - w1 - w2 + w3)

    def shard_tables(arr_t):
        """[.., T] -> per-core [.., PIX_PAD, TPP] zero-padded pixel grid."""
        shaped = arr_t.reshape(arr_t.shape[:-1] + (N_CORES, PIX_NC, TPP))
        pad = np.zeros(arr_t.shape[:-1] + (N_CORES, PIX_PAD - PIX_NC, TPP), arr_t.dtype)
        return np.concatenate([shaped, pad], axis=-2)

    # device layouts
    bf = ml_dtypes.bfloat16
    x0_s = shard_tables(x0)   # [B, NC, 1920, 144]
    x1_s = shard_tables(x1)
    cb_s = shard_tables(cb[None])[0]  # [NC, 1920, 144]
    cc_s = shard_tables(cc[None])[0]
    cd_s = shard_tables(cd[None])[0]
    ca_s = shard_tables(ca[None])[0]

    in_maps = []
    for n in range(N_CORES):
        # [B, 1920, 144] -> [B, 128, PPP*TPP] -> [128, B*PPP*TPP]
        def xlay(a):
            v = a[:, n].reshape(B, 128, TAB_PP).transpose(1, 0, 2)
            return np.ascontiguousarray(v.reshape(128, FREE)).astype(bf)

        def clay(a):
            v = a[n].reshape(128, TAB_PP).astype(bf)
            return np.ascontiguousarray(np.tile(v, (1, BG)))

        bias = ca_s[n].reshape(128, PPP, TPP).sum(axis=-1, dtype=np.float64)
        bias = np.ascontiguousarray(
            np.tile(bias.astype(np.float32), (1, BG))
        )
        in_maps.append(
            {
                "x0": xlay(x0_s),
                "x1": xlay(x1_s),
                "cb": clay(cb_s),
                "cc": clay(cc_s),
                "cd": clay(cd_s),
                "bias": bias,
            }
        )

    key = "nc"
    if key not in _NC_CACHE:
        _NC_CACHE[key] = _build_device_kernel()
    nc = _NC_CACHE[key]

    res = run_bass_kernel_spmd(nc, in_maps, core_ids=list(range(N_CORES)))

    # ---- unshard ----
    out = np.empty((B, OUT_CH, H_OUT, W_OUT), dtype=np.float32)
    for n in range(N_CORES):
        o = res.results[n]["out"]                    # [128, B*PPP]
        o = o.reshape(128, B, PPP).transpose(1, 0, 2).reshape(B, PIX_PAD)
        pix = o[:, :PIX_NC].reshape(B, 2, POS)
        out[:, 2 * n] = pix[:, 0].reshape(B, H_OUT, W_OUT)
        out[:, 2 * n + 1] = pix[:, 1].reshape(B, H_OUT, W_OUT)
    return out

